# revision 10
# baseline (speedup 1.0000x reference)
"""MixLinear quantized GEMM (nn_MixLinear_GEMM) — TRN2 Bass kernel, 8-core SPMD.

kernel(x, weight, bias) takes FULL inputs and returns the FULL output:
  x [2, 2048, 4096] f32, weight [4096, 4096] f32, bias [4096] f32
  -> y [2, 2048, 4096] f32

Reference math:
  col_mask[k] = any(|x[:,k]| > 20)   (over all 4096 rows)
  q_scale[n]  = max(1e-8, max_k |w[n,k]*keep[k]| / 127)
  row_scale[m]= max(1e-8, max_k |x[m,k]| / 127)
  y = round(clip(x/rs)) @ round(clip(w*keep/qs)).T * rs * qs + x*mask @ w.T + bias

Implemented as ONE fp16 GEMM with "hybrid" operands:
  A[m,k] = keep[k]*round(x[m,k]/rs[m]) + mask[k]*(x[m,k]/rs[m])   (fp16)
  B[n,k] = keep[k]*round(w[n,k]/qs[n]) + mask[k]*(w[n,k]/qs[n])   (fp16)
  y = (A @ B.T) * rs * qs + bias
Integer parts are exact in fp16 and accumulate exactly in f32 PSUM; the mask
columns reproduce the fp outlier-correction GEMM (fp16-rounded, ~2.5e-4 rel).

Sharding (8 cores): column-parallel GEMM — core c owns w rows / bias / y cols
[512c, 512c+512). Activation prep is row-sharded: core c quantizes+transposes
x rows [512c, 512c+512), then AllGathers A^T; the outlier column mask is a
16KB AllReduce(max) of per-core column maxima.
"""

import numpy as np

import concourse.bass as bass
import concourse.mybir as mybir
import concourse.tile as tile
from concourse import bacc
from concourse.bass_utils import run_bass_kernel_spmd
from concourse.masks import make_identity

F32 = mybir.dt.float32
F16 = mybir.dt.float16
I8 = mybir.dt.int8
ALU = mybir.AluOpType
AX = mybir.AxisListType

SIGMA = 20.0
EPS = 1e-8
MAGIC = 12582912.0  # 2^23 + 2^22: f32 add forces round-to-nearest-even to integer

B_, S_, K_, N_ = 2, 2048, 4096, 4096
M_ = B_ * S_
C_ = 8


def build_mixlinear(tc: tile.TileContext, outs, ins, *, M, K, N, C):
    nc = tc.nc
    MS, NS = M // C, N // C      # per-core shard sizes
    MT = MS // 128               # local m-tiles
    NT = NS // 128               # local w-row tiles
    KB = K // 128                # k blocks
    KG = KB // 4                 # k block groups of 4 (psum batching)
    assert MS % 128 == 0 and NS % 128 == 0 and K % 512 == 0 and KB % 4 == 0

    x_in = ins["x"]       # [MS, K]  f32
    w_in = ins["w"]       # [NS, K]  f32
    b_in = ins["bias"]    # [NS]     f32
    y_out = outs["y"]     # [M, NS]  f32

    rg = [list(range(C))]

    with (
        tc.tile_pool(name="dram", bufs=1, space="DRAM") as dram,
        tc.tile_pool(name="const", bufs=1) as const,
    ):
        # ---- internal DRAM for collectives ----
        cm_in = dram.tile([128, KB], F32, tag="cm_in")
        cm_out = dram.tile([128, KB], F32, tag="cm_out")
        at_in = [dram.tile([128, K], F16, tag=f"at_in{j}", name=f"at_in{j}")
                 for j in range(MT)]
        cc_space = "Shared" if C > 4 else "Local"
        at_out = [dram.tile([C, 128, K], F16, tag=f"at_out{j}", name=f"at_out{j}",
                            addr_space=cc_space)
                  for j in range(MT)]
        rs_in = dram.tile([128, MT], F32, tag="rs_in")
        rs_out = dram.tile([C, 128, MT], F32, tag="rs_out")
        mrow_dram = dram.tile([KB, 128], F32, tag="mrow")
        qrow_dram = dram.tile([NT, 128], F32, tag="qrow")

        # ---- persistent SBUF ----
        # identities staged through a DVE copy so PE consumers carry a single
        # (DVE) wait instead of {GPSIMD, DVE} pairs (TT/MM structs encode one
        # sync-wait command).
        ident32g = const.tile([128, 128], F32, tag="id32g")
        make_identity(nc, ident32g[:])
        ident32 = const.tile([128, 128], F32, tag="id32")
        nc.vector.tensor_copy(ident32[:], ident32g[:])
        ident16g = const.tile([128, 128], F16, tag="id16g")
        make_identity(nc, ident16g[:])
        ident16 = const.tile([128, 128], F16, tag="id16")
        nc.vector.tensor_copy(ident16[:], ident16g[:])
        mask_b = const.tile([128, K], F16, tag="mask_b")   # 1.0 on outlier cols
        keep_b = const.tile([128, K], F16, tag="keep_b")   # 1.0 on kept cols
        bt_sb = const.tile([128, KB, NS], F16, tag="bt")   # B^T, resident
        qsb = const.tile([128, NS], F32, tag="qsb")        # q_scale bcast over parts
        biasb = const.tile([128, NS], F32, tag="biasb")    # bias bcast over parts
        rs_all = const.tile([128, MT], F32, tag="rs_all")  # local row scales
        inv_all = const.tile([128, MT], F32, tag="inv_all")
        rs_full = const.tile([128, C, MT], F32, tag="rs_full")
        ones_col = const.tile([1, 128], F32, tag="ones")
        nc.vector.memset(ones_col[:], 1.0)

        # =========== Phase 1: x shard load, colmax, row scales, int8 ===========
        with (
            tc.tile_pool(name="xp", bufs=MT) as xp,
            tc.tile_pool(name="rc8p", bufs=MT) as rc8p,
            tc.tile_pool(name="p1s", bufs=2) as p1s,
        ):
            xts, rc8s = [], []
            with tc.tile_pool(name="p1tmp", bufs=1) as p1tmp, \
                 tc.tile_pool(name="p1rr", bufs=1) as p1rr, \
                 tc.tile_pool(name="p1ps", bufs=2, space="PSUM") as p1ps:
                cm = p1tmp.tile([128, K], F32, tag="cm")
                for i in range(MT):
                    xt = xp.tile([128, K], F32, tag="x")
                    nc.sync.dma_start(xt[:], x_in[i * 128:(i + 1) * 128, :])
                    xts.append(xt)
                    # |x| on ACT (abs_max is not a valid HW tensor_scalar op)
                    absx = p1rr.tile([128, K], F32, tag="absx")
                    nc.scalar.activation(absx[:], xt[:],
                                         mybir.ActivationFunctionType.Abs)
                    if i == 0:
                        nc.vector.tensor_copy(cm[:], absx[:])
                    else:
                        nc.vector.tensor_tensor(cm[:], cm[:], absx[:], ALU.max)
                    am = p1s.tile([128, 1], F32, tag="am")
                    nc.vector.tensor_reduce(am[:], absx[:], axis=AX.X, op=ALU.max)
                    nc.vector.tensor_scalar(rs_all[:, i:i + 1], am[:], 1.0 / 127.0,
                                            EPS, ALU.mult, ALU.max)
                    nc.vector.reciprocal(inv_all[:, i:i + 1], rs_all[:, i:i + 1])
                    rr = p1rr.tile([128, K], F32, tag="rr")
                    nc.vector.tensor_scalar(rr[:], xt[:], inv_all[:, i:i + 1], MAGIC,
                                            ALU.mult, ALU.add)
                    rc8 = rc8p.tile([128, K], I8, tag="rc8")
                    nc.vector.tensor_scalar(rc8[:], rr[:], MAGIC, None, ALU.subtract)
                    rc8s.append(rc8)

                # partition-reduce colmax: per k-block transpose + free reduce
                cmT = p1tmp.tile([128, KB], F32, tag="cmT")
                for kb in range(KB):
                    pt = p1ps.tile([128, 512], F32, tag="ps1")
                    nc.tensor.transpose(pt[:, 0:128], cm[:, kb * 128:(kb + 1) * 128],
                                        ident32[:])
                    nc.vector.tensor_reduce(cmT[:, kb:kb + 1], pt[:, 0:128],
                                            axis=AX.X, op=ALU.max)

                # global colmax via AllReduce(max)
                nc.sync.dma_start(cm_in[:], cmT[:])
                nc.gpsimd.collective_compute("AllReduce", ALU.max, replica_groups=rg,
                                             ins=[cm_in.opt()], outs=[cm_out.opt()])
                cmg = p1tmp.tile([128, KB], F32, tag="cmg")
                nc.sync.dma_start(cmg[:], cm_out[:])

                # mask row in k-order: is_gt -> transpose -> [KB,128] -> DRAM -> [1,K]
                mkp = p1tmp.tile([128, KB], F32, tag="mkp")
                nc.vector.tensor_scalar(mkp[:], cmg[:], SIGMA, None, ALU.is_gt)
                mps = p1ps.tile([KB, 128], F32, tag="mps")
                nc.tensor.transpose(mps[:], mkp[:], ident32[:])
                msb = p1tmp.tile([KB, 128], F32, tag="msb")
                nc.vector.tensor_copy(msb[:], mps[:])
                nc.sync.dma_start(mrow_dram[:], msb[:])
                # broadcast mask row across 128 partitions via K=1 matmuls,
                # in [1,512] chunks ([1,K] tiles would cost 16KB/partition)
                mrow_flat = mrow_dram[:].rearrange("a b -> (a b)")
                for nb in range(K // 512):
                    mrow_d = p1tmp.tile([1, 512], F32, tag="mrow_d", bufs=2)
                    nc.sync.dma_start(mrow_d[:],
                                      mrow_flat[nb * 512:(nb + 1) * 512][None, :])
                    mrow = p1tmp.tile([1, 512], F32, tag="mrow", bufs=2)
                    nc.vector.tensor_copy(mrow[:], mrow_d[:])  # DVE-stage for PE
                    bps = p1ps.tile([128, 512], F32, tag="ps1")
                    nc.tensor.matmul(bps[:], ones_col[:], mrow[:],
                                     start=True, stop=True)
                    nc.vector.tensor_copy(mask_b[:, nb * 512:(nb + 1) * 512], bps[:])
                nc.vector.tensor_scalar(keep_b[:], mask_b[:], -1.0, 1.0,
                                        ALU.mult, ALU.add)

            # =========== Phase 2: hybrid A, transpose, AllGather ===========
            nc.sync.dma_start(rs_in[:], rs_all[:])
            nc.gpsimd.collective_compute("AllGather", ALU.bypass, replica_groups=rg,
                                         ins=[rs_in.opt()], outs=[rs_out.opt()])

            with (
                tc.tile_pool(name="tmp2", bufs=1) as tmp2,
                tc.tile_pool(name="ap", bufs=2) as ap,
                tc.tile_pool(name="atp", bufs=2) as atp,
                tc.tile_pool(name="p2ps", bufs=2, space="PSUM") as p2ps,
            ):
                for i in range(MT):
                    tm = tmp2.tile([128, K], F16, tag="tm")
                    nc.vector.scalar_tensor_tensor(tm[:], xts[i][:],
                                                   inv_all[:, i:i + 1],
                                                   mask_b[:], ALU.mult, ALU.mult)
                    ah = ap.tile([128, K], F16, tag="ah")
                    nc.vector.scalar_tensor_tensor(ah[:], rc8s[i][:], 1.0, keep_b[:],
                                                   ALU.mult, ALU.mult)
                    nc.vector.tensor_tensor(ah[:], ah[:], tm[:], ALU.add)
                    at_sb = atp.tile([128, K], F16, tag="at_sb")
                    for g in range(KG):
                        pst = p2ps.tile([128, 512], F16, tag="pst")
                        for q in range(4):
                            kb = g * 4 + q
                            nc.tensor.transpose(pst[:, q * 128:(q + 1) * 128],
                                                ah[:, kb * 128:(kb + 1) * 128],
                                                ident16[:])
                        nc.vector.tensor_copy(at_sb[:, g * 512:(g + 1) * 512], pst[:])
                    nc.sync.dma_start(at_in[i][:], at_sb[:])
                    nc.gpsimd.collective_compute("AllGather", ALU.bypass,
                                                 replica_groups=rg,
                                                 ins=[at_in[i].opt()],
                                                 outs=[at_out[i].opt()])

        # =========== Phase 2b: hybrid B^T (resident), q scales ===========
        with (
            tc.tile_pool(name="wp", bufs=2) as wp,
            tc.tile_pool(name="wsc", bufs=1) as wsc,
            tc.tile_pool(name="bp", bufs=2) as bp,
            tc.tile_pool(name="p3s", bufs=1) as p3s,
            tc.tile_pool(name="p3ps", bufs=2, space="PSUM") as p3ps,
        ):
            qs_all = p3s.tile([128, NT], F32, tag="qs_all")
            for wi in range(NT):
                wt = wp.tile([128, K], F32, tag="w")
                nc.sync.dma_start(wt[:], w_in[wi * 128:(wi + 1) * 128, :])
                wm = wsc.tile([128, K], F32, tag="wm")
                nc.vector.tensor_tensor(wm[:], wt[:], keep_b[:], ALU.mult)
                qm = p3s.tile([128, 1], F32, tag="qm")
                nc.vector.tensor_reduce(qm[:], wm[:], axis=AX.X, op=ALU.max,
                                        apply_absolute_value=True)
                nc.vector.tensor_scalar(qs_all[:, wi:wi + 1], qm[:], 1.0 / 127.0,
                                        EPS, ALU.mult, ALU.max)
                invq = p3s.tile([128, 1], F32, tag="invq")
                nc.vector.reciprocal(invq[:], qs_all[:, wi:wi + 1])
                wrr = wsc.tile([128, K], F32, tag="wrr")
                nc.vector.tensor_scalar(wrr[:], wt[:], invq[:], MAGIC,
                                        ALU.mult, ALU.add)
                rcw = wsc.tile([128, K], I8, tag="rcw")
                nc.vector.tensor_scalar(rcw[:], wrr[:], MAGIC, None, ALU.subtract)
                twm = wsc.tile([128, K], F16, tag="twm")
                nc.vector.scalar_tensor_tensor(twm[:], wt[:], invq[:], mask_b[:],
                                               ALU.mult, ALU.mult)
                bh = bp.tile([128, K], F16, tag="bh")
                nc.vector.scalar_tensor_tensor(bh[:], rcw[:], 1.0, keep_b[:],
                                               ALU.mult, ALU.mult)
                nc.vector.tensor_tensor(bh[:], bh[:], twm[:], ALU.add)
                for g in range(KG):
                    pst = p3ps.tile([128, 512], F16, tag="wpst")
                    for q in range(4):
                        kb = g * 4 + q
                        nc.tensor.transpose(pst[:, q * 128:(q + 1) * 128],
                                            bh[:, kb * 128:(kb + 1) * 128],
                                            ident16[:])
                    nc.vector.tensor_copy(
                        bt_sb[:, g * 4:(g + 1) * 4, wi * 128:(wi + 1) * 128],
                        pst[:].rearrange("p (a b) -> p a b", a=4))

            # q_scale row -> [1, NS]; bias row; broadcast both over partitions
            qps = p3ps.tile([NT, 128], F32, tag="qps")
            nc.tensor.transpose(qps[:], qs_all[:], ident32[:])
            qsrow = p3s.tile([NT, 128], F32, tag="qsrow")
            nc.vector.tensor_copy(qsrow[:], qps[:])
            nc.sync.dma_start(qrow_dram[:], qsrow[:])
            qrow_d = p3s.tile([1, NS], F32, tag="qrow_d")
            nc.sync.dma_start(qrow_d[:],
                              qrow_dram[:].rearrange("a b -> (a b)")[None, :])
            qrow = p3s.tile([1, NS], F32, tag="qrow")
            nc.vector.tensor_copy(qrow[:], qrow_d[:])  # DVE-stage for PE
            brow_d = p3s.tile([1, NS], F32, tag="brow_d")
            nc.sync.dma_start(brow_d[:], b_in[None, :])
            brow = p3s.tile([1, NS], F32, tag="brow")
            nc.vector.tensor_copy(brow[:], brow_d[:])  # DVE-stage for PE
            for nb in range((NS + 511) // 512):
                w0 = nb * 512
                w1 = min(NS, w0 + 512)
                bps = p3ps.tile([128, 512], F32, tag="qbps")
                nc.tensor.matmul(bps[:, 0:w1 - w0], ones_col[:], qrow[:, w0:w1],
                                 start=True, stop=True)
                nc.vector.tensor_copy(qsb[:, w0:w1], bps[:, 0:w1 - w0])
                bps2 = p3ps.tile([128, 512], F32, tag="qbps")
                nc.tensor.matmul(bps2[:, 0:w1 - w0], ones_col[:], brow[:, w0:w1],
                                 start=True, stop=True)
                nc.vector.tensor_copy(biasb[:, w0:w1], bps2[:, 0:w1 - w0])

        # gather rs from all cores; DVE-stage so epilogue stt waits on PE only
        rs_full_d = const.tile([128, C, MT], F32, tag="rs_full_d")
        for c in range(C):
            nc.sync.dma_start(rs_full_d[:, c, :], rs_out[c])
        nc.vector.tensor_copy(rs_full[:], rs_full_d[:])

        # =========== Phase 4: GEMM + epilogue ===========
        with (
            tc.tile_pool(name="gat", bufs=3) as gat,
            tc.tile_pool(name="gy", bufs=2) as gy,
            tc.tile_pool(name="gps", bufs=2, space="PSUM") as gps,
        ):
            for j in range(MT):
                for c in range(C):
                    att = gat.tile([128, K], F16, tag="att")
                    nc.sync.dma_start(att[:], at_out[j][c])
                    psy = gps.tile([128, NS], F32, tag="psy")
                    for kb in range(KB):
                        nc.tensor.matmul(psy[:], att[:, kb * 128:(kb + 1) * 128],
                                         bt_sb[:, kb, :],
                                         start=(kb == 0), stop=(kb == KB - 1))
                    y1 = gy.tile([128, NS], F32, tag="y1")
                    nc.vector.scalar_tensor_tensor(y1[:], psy[:],
                                                   rs_full[:, c, j:j + 1],
                                                   qsb[:], ALU.mult, ALU.mult)
                    nc.vector.tensor_tensor(y1[:], y1[:], biasb[:], ALU.add)
                    row0 = c * MS + j * 128
                    nc.sync.dma_start(y_out[row0:row0 + 128, :], y1[:])


_CACHED = None


def _build():
    global _CACHED
    if _CACHED is not None:
        return _CACHED
    MS, NS = M_ // C_, N_ // C_
    nc = bacc.Bacc("TRN2", target_bir_lowering=False, debug=False,
                   num_devices=C_)
    ins = {
        "x": nc.dram_tensor("x", [MS, K_], F32, kind="ExternalInput").ap(),
        "w": nc.dram_tensor("w", [NS, K_], F32, kind="ExternalInput").ap(),
        "bias": nc.dram_tensor("bias", [NS], F32, kind="ExternalInput").ap(),
    }
    outs = {
        "y": nc.dram_tensor("y", [M_, NS], F32, kind="ExternalOutput").ap(),
    }
    with tile.TileContext(nc) as tc:
        build_mixlinear(tc, outs, ins, M=M_, K=K_, N=N_, C=C_)
    nc.compile()
    _CACHED = nc
    return nc


def run_spmd(x, weight, bias, **spmd_kwargs):
    """Run the SPMD kernel; returns (y_full, BassKernelResults)."""
    nc = _build()
    MS, NS = M_ // C_, N_ // C_
    xf = np.ascontiguousarray(np.asarray(x, np.float32).reshape(M_, K_))
    w = np.ascontiguousarray(np.asarray(weight, np.float32))
    b = np.ascontiguousarray(np.asarray(bias, np.float32))
    in_maps = [
        {
            "x": xf[c * MS:(c + 1) * MS],
            "w": w[c * NS:(c + 1) * NS],
            "bias": b[c * NS:(c + 1) * NS],
        }
        for c in range(C_)
    ]
    res = run_bass_kernel_spmd(nc, in_maps, core_ids=list(range(C_)), **spmd_kwargs)
    y = np.concatenate([res.results[c]["y"] for c in range(C_)], axis=1)
    return y.reshape(B_, S_, N_), res


def kernel(x, weight, bias):
    y, _ = run_spmd(x, weight, bias)
    return y
